# revision 20
# baseline (speedup 1.0000x reference)
"""LocallyConnected2d Trainium2 kernel (bf16 pipeline).

Problem: out[b,o,oh,ow] = sum_{c,ki,kj} x[b,c,oh+ki,ow+kj] * W[o,oh,ow,c,ki,kj] + bias[o,oh,ow]
Shapes: x[32,32,64,64], W[64,62,62,32,3,3], bias[64,62,62] -> out[32,64,62,62], fp32 I/O.

The untied weight tensor (283 MB fp32) is read exactly once -> the kernel is
HBM-bandwidth bound. All operands ship as bf16 (accuracy gate 2e-2 vs ~2e-3
bf16 quantization error), halving the dominant weight stream; PSUM accumulates
in fp32; the output returns as bf16 and is upcast on host.

Strategy (8 NeuronCores, sharded over output rows, 8 rows/core padded to 64):
- Per output location: 3 accumulating PE matmuls, K=97 each (chunk q = kernel
  row ki; features j=(kj,c) plus a ones-row at j=96 that carries bias on q=2).
- lhsT (stationary) = x patch columns [97,32b]: x is loaded into SBUF once as
  3 column-shifted replicas on partitions kj*32+c, so every lhsT is a direct
  AP slice (no im2col data movement). Partition 96 = constant 1.0.
- rhs (moving) = per-location weights [97,64o], streamed from HBM in
  half-row strips with a host-side layout [row, half, j, q, ow, o] making each
  strip one fully-contiguous DMA (97 x 11.9KB descriptors).
- One PSUM bank [128,512] per strip accumulates 8 location-groups (4 locations
  x 32b on partitions, 64o per group in free); a single DVE copy casts the
  bank to a bf16 SBUF strip; one contiguous DMA per half-row out.
"""

import numpy as np
import ml_dtypes

import concourse.bass as bass  # noqa: F401
import concourse.mybir as mybir
import concourse.tile as tile
from concourse import bacc
from concourse.bass_utils import run_bass_kernel_spmd

B, C_IN, H, W = 32, 32, 64, 64
C_OUT, OH, OW, KK = 64, 62, 62, 3
N_CORES = 8
ROWS = 8          # padded output rows per core (8*8=64 >= 62)
HALF = 31         # locations per strip (half an output row)
XH = ROWS + 2     # input rows needed per core
KP = 97           # contraction per chunk: 96 features + ones/bias row
NG = 8            # ceil(31/4) location groups per strip
F32 = mybir.dt.float32
BF16 = mybir.dt.bfloat16
NP_BF16 = ml_dtypes.bfloat16

_NC_CACHE = {}


def _build_nc():
    nc = bacc.Bacc(
        "TRN2",
        target_bir_lowering=False,
        debug=False,
        enable_asserts=False,
        num_devices=N_CORES,
    )
    # x ships host-transposed AND pre-shifted into 3 kj-replicas
    # [kj, c, h, w(62), b] so the whole x3 load is one contiguous DMA
    x_d = nc.dram_tensor("x", [KK, C_IN, XH, OW, B], BF16, kind="ExternalInput").ap()
    # w ships regrouped by 4-strip groups: [group, j, 4 x (q l o) + pad].
    # Per-engine SDMA throughput is descriptor-length bound (~330ns setup +
    # bytes/27GB/s): 11.9KB lines gave 15.6 GB/s/engine; 47.6KB lines (one
    # line spans 4 strips) push toward line rate. The 32-elem pad keeps the
    # source non-contiguous so descriptors don't concat-collapse onto one
    # engine.
    SLINE = 3 * HALF * C_OUT  # 5952 elems per strip per j-line
    GLINE = 4 * SLINE + 32  # 23840
    w_d = nc.dram_tensor("w", [4, KP, GLINE], BF16, kind="ExternalInput").ap()
    ones_d = nc.dram_tensor("ones", [1, XH * OW * B], BF16, kind="ExternalInput").ap()
    # out layout: [p=(l4,b), strip, grp, o] - partition-major so each store
    # DMA covers several strips with fat contiguous per-partition lines;
    # host unscrambles + upcasts
    o_d = nc.dram_tensor(
        "out", [128, ROWS * 2 * NG * C_OUT], BF16, kind="ExternalOutput"
    ).ap()

    with tile.TileContext(nc) as tc:
        with (
            tc.tile_pool(name="xpool", bufs=1) as xpool,
            tc.tile_pool(name="wpool", bufs=2) as wpool,
            tc.tile_pool(name="opool", bufs=1) as opool,
            tc.tile_pool(name="pspool", bufs=3, space="PSUM") as pspool,
        ):
            # x replicas: partition kj*32+c holds x[b,c,h,w+kj] at free
            # (h, w, b); partition 96 = 1.0 (carries the bias row).
            # SWDGE (gpsimd) sprays each partition line into 16 tiny
            # descriptors (~97 GB/s measured); everything rides the two
            # HWDGE rings (sync=weights, scalar=x tail + out) instead.
            HZ = OW * B  # 1984
            x3 = xpool.tile([KP, XH * HZ], BF16)
            xsrc = x_d.rearrange("k c h w b -> (k c) (h w b)")

            def load_x_rows(r0, r1, eng):
                eng.dma_start(
                    out=x3[0:96, r0 * HZ : r1 * HZ],
                    in_=xsrc[0:96, r0 * HZ : r1 * HZ],
                )

            # rows 0-2 feed strip 0; they go ahead of the w strips on the
            # sync ring. Rows 3-9 + ones ride the scalar ring concurrently.
            load_x_rows(0, 3, nc.sync)
            nc.scalar.dma_start(out=x3[96:97, :], in_=ones_d)
            load_x_rows(3, 6, nc.scalar)
            load_x_rows(6, 10, nc.scalar)

            QZ = HALF * C_OUT  # 1984, one chunk per kernel row q
            ot = opool.tile([128, ROWS * 2 * NG * C_OUT], BF16)  # all strips

            # HWDGE fans a DMA across the 16 SDMA engines only when the
            # outer (line) dim is divisible by 16 — a 97-line transfer lands
            # on ONE engine. Split each group into the 96 feature lines
            # (spreads 16-way) and the single bias line. Groups alternate
            # between the two HWDGE rings; group G+1 is issued before group
            # G's out-store so stores never head-of-line-block the stream.
            def load_w_group(G, fine):
                weng = nc.sync if G % 2 == 0 else nc.scalar
                wt_full = wpool.tile([KP, GLINE], BF16, tag="wt")
                wsrc = w_d[G]
                # fine: per-strip loads so the first strip's matmuls unblock
                # after 1/4 of the group
                chunks = (
                    [(s * SLINE, (s + 1) * SLINE) for s in range(4)]
                    if fine
                    else [(0, 4 * SLINE)]
                )
                for f0, f1 in chunks:
                    weng.dma_start(out=wt_full[96:97, f0:f1], in_=wsrc[96:97, f0:f1])
                    weng.dma_start(out=wt_full[0:96, f0:f1], in_=wsrc[0:96, f0:f1])
                return wt_full

            wt_next = load_w_group(0, fine=True)
            for G in range(4):  # 4-strip weight groups
                wt_full = wt_next
                if G + 1 < 4:
                    wt_next = load_w_group(G + 1, fine=False)
                for s4 in range(4):
                    strip = G * 4 + s4
                    row, half = strip // 2, strip % 2
                    wt = wt_full[:, s4 * SLINE : (s4 + 1) * SLINE]
                    # one PSUM bank for the whole strip: partitions (l4,b),
                    # free (grp, o)
                    ps = pspool.tile([128, NG * C_OUT], F32, tag="ps")
                    for g in range(NG):
                        gn = min(4, HALF - g * 4)  # 4,4,...,3
                        for li in range(4):
                            # pad slot in the last group duplicates the prior
                            # location (keeps PSUM fully written; host drops it)
                            eff = min(li, gn - 1)
                            ow = half * HALF + g * 4 + eff
                            loff = (g * 4 + eff) * C_OUT
                            for q in range(3):
                                nc.tensor.matmul(
                                    ps[32 * li : 32 * li + 32, g * C_OUT : (g + 1) * C_OUT],
                                    x3[
                                        :,
                                        (row + q) * HZ
                                        + ow * B : (row + q) * HZ
                                        + ow * B
                                        + B,
                                    ],  # [97, 32] lhsT
                                    wt[:, q * QZ + loff : q * QZ + loff + C_OUT],
                                    start=(q == 0),
                                    stop=(q == 2),
                                    tile_position=(0, 32 * li),
                                )
                    SZ = NG * C_OUT  # 512 out elems per strip per partition
                    nc.vector.tensor_copy(
                        out=ot[:, strip * SZ : (strip + 1) * SZ], in_=ps
                    )
                # store per group: 4KB contiguous per-partition lines on the
                # ring opposite the NEXT group's weight stream
                oeng = nc.scalar if G % 2 == 0 else nc.sync
                c0, c1 = G * 4 * 512, (G + 1) * 4 * 512
                oeng.dma_start(out=o_d[:, c0:c1], in_=ot[:, c0:c1])

    nc.compile()
    return nc


def get_nc():
    if "nc" not in _NC_CACHE:
        _NC_CACHE["nc"] = _build_nc()
    return _NC_CACHE["nc"]


def prep_inputs(x, weight, bias):
    """Host-side shard + layout prep. Returns per-core in_maps."""
    x = np.asarray(x, dtype=np.float32)
    weight = np.asarray(weight, dtype=np.float32)
    bias = np.asarray(bias, dtype=np.float32)

    # w_prep[oh, j=kj*32+c, q=ki, ow, o]; j=96 row: 0 for q<2, bias for q=2
    wp = np.zeros((N_CORES * ROWS, KP, 3, OW, C_OUT), NP_BF16)
    wp[:OH, :96] = (
        weight.transpose(1, 5, 3, 4, 2, 0).reshape(OH, 96, 3, OW, C_OUT)
    ).astype(NP_BF16)
    wp[:OH, 96, 2] = bias.transpose(1, 2, 0).astype(NP_BF16)
    # half-row strips: [row, half, j, (q l o)] -> per-core 4-strip groups
    # with one padded 4-strip-wide line per j: [core, group, j, 4*(q l o)+32]
    sline = 3 * HALF * C_OUT  # 5952
    wp = wp.reshape(N_CORES * ROWS, KP, 3, 2, HALF, C_OUT).transpose(0, 3, 1, 2, 4, 5)
    wp = wp.reshape(N_CORES, 2 * ROWS, KP, sline)  # [core, strip, j, sline]
    wp = wp.reshape(N_CORES, 4, 4, KP, sline).transpose(0, 1, 3, 2, 4)
    gline = 4 * sline + 32
    wpad = np.zeros((N_CORES, 4, KP, gline), NP_BF16)
    wpad[:, :, :, : 4 * sline] = wp.reshape(N_CORES, 4, KP, 4 * sline)
    wp = wpad

    xp = np.zeros((B, C_IN, N_CORES * ROWS + 2, W), NP_BF16)
    xp[:, :, :H] = x.astype(NP_BF16)
    xt = xp.transpose(1, 2, 3, 0)  # [c, h, w, b]

    ones = np.ones((1, XH * OW * B), NP_BF16)

    in_maps = []
    for c in range(N_CORES):
        r0 = c * ROWS
        xc = xt[:, r0 : r0 + XH]  # [c, 10, 64, b]
        xsh = np.stack([xc[:, :, kj : kj + OW, :] for kj in range(KK)])
        in_maps.append(
            {
                "x": np.ascontiguousarray(xsh),
                "w": np.ascontiguousarray(wp[c]),
                "ones": ones,
            }
        )
    return in_maps


def gather_output(results):
    """results: list of per-core out dicts -> full [B, C_OUT, OH, OW] fp32."""
    out = np.empty((B, C_OUT, OH, OW), np.float32)
    for c in range(N_CORES):
        # out[p=(l4,b), (row, half, grp, o)]
        oc = np.asarray(results[c]["out"]).astype(np.float32)
        v = oc.reshape(4, B, ROWS, 2, NG, C_OUT)
        # ow = half*31 + grp*4 + l  (grp*4+l < 31)
        arr = v.transpose(1, 5, 2, 3, 4, 0).reshape(B, C_OUT, ROWS, 2, 32)
        arr = arr[:, :, :, :, :HALF].reshape(B, C_OUT, ROWS, OW)
        r0 = c * ROWS
        rows = min(ROWS, OH - r0)
        out[:, :, r0 : r0 + rows, :] = arr[:, :, :rows, :]
    return out


def run(inputs, **kw):
    nc = get_nc()
    in_maps = prep_inputs(inputs["x"], inputs["weight"], inputs["bias"])
    res = run_bass_kernel_spmd(nc, in_maps, core_ids=list(range(N_CORES)), **kw)
    return gather_output(res.results), res


def kernel(x, weight, bias):
    out, _ = run({"x": x, "weight": weight, "bias": bias})
    return out


# revision 21
# speedup vs baseline: 1.0342x; 1.0342x over previous
"""LocallyConnected2d Trainium2 kernel (bf16 pipeline).

Problem: out[b,o,oh,ow] = sum_{c,ki,kj} x[b,c,oh+ki,ow+kj] * W[o,oh,ow,c,ki,kj] + bias[o,oh,ow]
Shapes: x[32,32,64,64], W[64,62,62,32,3,3], bias[64,62,62] -> out[32,64,62,62], fp32 I/O.

The untied weight tensor (283 MB fp32) is read exactly once -> the kernel is
HBM-bandwidth bound. All operands ship as bf16 (accuracy gate 2e-2 vs ~2.7e-3
measured bf16 error), halving the dominant weight stream; PSUM accumulates in
fp32; the output returns as bf16 and is upcast on host.

Strategy (8 NeuronCores, sharded over output rows, 8 rows/core padded to 64):
- Per output location: 3 accumulating PE matmuls, K=97 each (chunk q = kernel
  row ki; features j=(kj,c) plus a ones-row at j=96 that carries bias on q=2).
- lhsT (stationary) = x patch columns [97,32b]: x ships unshifted [c,h,w64,b]
  (1.3 MB) and is replicated on-chip into 3 column-shifted replicas on
  partitions kj*32+c via SBUF->SBUF DMA, so every lhsT is a direct AP slice.
  Partition 96 is memset to 1.0 (carries the bias row).
- rhs (moving) = per-location weights [97,64o], streamed from HBM in
  half-row strips (11.9KB padded lines). DMA shape lessons (measured):
  SWDGE chops lines 16-way into ~744B packets (~97 GB/s); HWDGE fans a DMA
  across the 16 SDMA engines only when the line count is divisible by 16,
  else the whole transfer lands on ONE engine (~27 GB/s). So: 96-line
  feature DMAs + separate bias line, alternating between the two HWDGE
  rings (sync/scalar), all weight DMAs emitted before any out-store.
- One PSUM bank [128,512] per strip accumulates 8 location-groups (4
  locations x 32b on partitions, 64o per group in free); one DVE copy casts
  the bank into a persistent bf16 out tile; out-stores go in multi-strip
  chunks with fat per-partition lines.
"""

import numpy as np
import ml_dtypes

import concourse.bass as bass  # noqa: F401
import concourse.mybir as mybir
import concourse.tile as tile
from concourse import bacc
from concourse.bass_utils import run_bass_kernel_spmd

B, C_IN, H, W = 32, 32, 64, 64
C_OUT, OH, OW, KK = 64, 62, 62, 3
N_CORES = 8
ROWS = 8          # padded output rows per core (8*8=64 >= 62)
HALF = 31         # locations per strip (half an output row)
XH = ROWS + 2     # input rows needed per core
KP = 97           # contraction per chunk: 96 features + ones/bias row
NG = 8            # ceil(31/4) location groups per strip
SLINE = 3 * HALF * C_OUT  # 5952 weight elems per strip per j-line
WLINE = SLINE + 32        # padded line (non-contiguous source)
F32 = mybir.dt.float32
BF16 = mybir.dt.bfloat16
NP_BF16 = ml_dtypes.bfloat16

_NC_CACHE = {}


def _build_nc():
    nc = bacc.Bacc(
        "TRN2",
        target_bir_lowering=False,
        debug=False,
        enable_asserts=False,
        num_devices=N_CORES,
    )
    x_d = nc.dram_tensor("x", [C_IN, XH, W, B], BF16, kind="ExternalInput").ap()
    w_d = nc.dram_tensor(
        "w", [ROWS, 2, KP, WLINE], BF16, kind="ExternalInput"
    ).ap()
    # out layout: [p=(l4,b), strip, grp, o] - partition-major; host
    # unscrambles + upcasts
    o_d = nc.dram_tensor(
        "out", [128, ROWS * 2 * NG * C_OUT], BF16, kind="ExternalOutput"
    ).ap()

    with tile.TileContext(nc) as tc:
        with (
            tc.tile_pool(name="xpool", bufs=1) as xpool,
            tc.tile_pool(name="wpool", bufs=6) as wpool,
            tc.tile_pool(name="opool", bufs=1) as opool,
            tc.tile_pool(name="pspool", bufs=3, space="PSUM") as pspool,
        ):
            HZ = OW * B    # 1984 elems per h-row in the shifted replicas
            HZ64 = W * B   # 2048 elems per h-row unshifted
            x1 = xpool.tile([C_IN, XH * HZ64], BF16)
            x3 = xpool.tile([KP, XH * HZ], BF16)
            nc.vector.memset(x3[96:97, :], 1.0)
            xsrc = x_d.rearrange("c h w b -> c (h w b)")
            # rows 0-2(+) ahead of the weight stream on sync; rest on scalar
            nc.sync.dma_start(out=x1[:, 0 : 3 * HZ64], in_=xsrc[:, 0 : 3 * HZ64])
            nc.scalar.dma_start(out=x1[:, 3 * HZ64 :], in_=xsrc[:, 3 * HZ64 :])
            # on-chip kj-replication: partition kj*32+c <- x[c, h, w+kj, b]
            # (SBUF->SBUF on gpsimd; saves 2.5 MB of HBM vs shipping 3
            # pre-shifted replicas)
            x1v = x1.rearrange("c (h z) -> c h z", z=HZ64)
            x3v = x3.rearrange("p (h z) -> p h z", z=HZ)
            for r0, r1 in ((0, 3), (3, XH)):
                for kj in range(KK):
                    nc.gpsimd.dma_start(
                        out=x3v[kj * 32 : (kj + 1) * 32, r0:r1, :],
                        in_=x1v[:, r0:r1, kj * B : kj * B + HZ],
                    )

            QZ = HALF * C_OUT  # 1984, one chunk per kernel row q
            # all weight DMAs emitted up-front (wpool bufs provide the
            # streaming backpressure) so out-stores never head-of-line
            # block the weight stream on either ring
            wts = []
            for s in range(2 * ROWS):
                weng = nc.sync if s % 2 == 0 else nc.scalar
                wt_full = wpool.tile([KP, WLINE], BF16, tag="wt")
                wsrc = w_d[s // 2, s % 2]
                weng.dma_start(
                    out=wt_full[96:97, 0:SLINE], in_=wsrc[96:97, 0:SLINE]
                )
                if s < 2:
                    # split the first strip on each ring by q-chunk so its
                    # first matmuls unblock after 1/3 of the strip
                    for f0, f1 in ((0, QZ), (QZ, 2 * QZ), (2 * QZ, 3 * QZ)):
                        weng.dma_start(
                            out=wt_full[0:96, f0:f1], in_=wsrc[0:96, f0:f1]
                        )
                else:
                    weng.dma_start(
                        out=wt_full[0:96, 0:SLINE], in_=wsrc[0:96, 0:SLINE]
                    )
                wts.append(wt_full)

            SZ = NG * C_OUT  # 512 out elems per strip per partition
            ot = opool.tile([128, 2 * ROWS * SZ], BF16)  # all strips
            OUT_CHUNKS = {3: (0, 4), 7: (4, 8), 11: (8, 12), 14: (12, 15), 15: (15, 16)}
            for s in range(2 * ROWS):
                row, half = s // 2, s % 2
                wt = wts[s]
                # one PSUM bank per strip: partitions (l4,b), free (grp, o)
                ps = pspool.tile([128, SZ], F32, tag="ps")
                for g in range(NG):
                    gn = min(4, HALF - g * 4)  # 4,4,...,3
                    for li in range(4):
                        # pad slot in the last group duplicates the prior
                        # location (keeps PSUM fully written; host drops it)
                        eff = min(li, gn - 1)
                        ow = half * HALF + g * 4 + eff
                        loff = (g * 4 + eff) * C_OUT
                        for q in range(3):
                            nc.tensor.matmul(
                                ps[32 * li : 32 * li + 32, g * C_OUT : (g + 1) * C_OUT],
                                x3[
                                    :,
                                    (row + q) * HZ
                                    + ow * B : (row + q) * HZ
                                    + ow * B
                                    + B,
                                ],  # [97, 32] lhsT
                                wt[:, q * QZ + loff : q * QZ + loff + C_OUT],
                                start=(q == 0),
                                stop=(q == 2),
                                tile_position=(0, 32 * li),
                            )
                nc.vector.tensor_copy(out=ot[:, s * SZ : (s + 1) * SZ], in_=ps)
                if s in OUT_CHUNKS:
                    c0, c1 = OUT_CHUNKS[s]
                    nc.scalar.dma_start(
                        out=o_d[:, c0 * SZ : c1 * SZ], in_=ot[:, c0 * SZ : c1 * SZ]
                    )

    nc.compile()
    return nc


def get_nc():
    if "nc" not in _NC_CACHE:
        _NC_CACHE["nc"] = _build_nc()
    return _NC_CACHE["nc"]


def prep_inputs(x, weight, bias):
    """Host-side shard + layout prep. Returns per-core in_maps."""
    x = np.asarray(x, dtype=np.float32)
    weight = np.asarray(weight, dtype=np.float32)
    bias = np.asarray(bias, dtype=np.float32)

    # w_prep[oh, j=kj*32+c, q=ki, ow, o]; j=96 row: 0 for q<2, bias for q=2
    wp = np.zeros((N_CORES * ROWS, KP, 3, OW, C_OUT), NP_BF16)
    wp[:OH, :96] = (
        weight.transpose(1, 5, 3, 4, 2, 0).reshape(OH, 96, 3, OW, C_OUT)
    ).astype(NP_BF16)
    wp[:OH, 96, 2] = bias.transpose(1, 2, 0).astype(NP_BF16)
    # half-row strips with padded lines: [row, half, j, (q l o)+32]
    wp = wp.reshape(N_CORES * ROWS, KP, 3, 2, HALF, C_OUT).transpose(0, 3, 1, 2, 4, 5)
    wpad = np.zeros((N_CORES * ROWS, 2, KP, WLINE), NP_BF16)
    wpad[:, :, :, :SLINE] = wp.reshape(N_CORES * ROWS, 2, KP, SLINE)
    wp = wpad

    # x unshifted [c, h, w64, b] per core (replication happens on-chip)
    xp = np.zeros((C_IN, N_CORES * ROWS + 2, W, B), NP_BF16)
    xp[:, :H] = x.transpose(1, 2, 3, 0).astype(NP_BF16)

    in_maps = []
    for c in range(N_CORES):
        r0 = c * ROWS
        in_maps.append(
            {
                "x": np.ascontiguousarray(xp[:, r0 : r0 + XH]),
                "w": np.ascontiguousarray(wp[r0 : r0 + ROWS]),
            }
        )
    return in_maps


def gather_output(results):
    """results: list of per-core out dicts -> full [B, C_OUT, OH, OW] fp32."""
    out = np.empty((B, C_OUT, OH, OW), np.float32)
    for c in range(N_CORES):
        # out[p=(l4,b), (strip, grp, o)]
        oc = np.asarray(results[c]["out"]).astype(np.float32)
        v = oc.reshape(4, B, ROWS, 2, NG, C_OUT)
        # ow = half*31 + grp*4 + l  (grp*4+l < 31)
        arr = v.transpose(1, 5, 2, 3, 4, 0).reshape(B, C_OUT, ROWS, 2, 32)
        arr = arr[:, :, :, :, :HALF].reshape(B, C_OUT, ROWS, OW)
        r0 = c * ROWS
        rows = min(ROWS, OH - r0)
        out[:, :, r0 : r0 + rows, :] = arr[:, :, :rows, :]
    return out


def run(inputs, **kw):
    nc = get_nc()
    in_maps = prep_inputs(inputs["x"], inputs["weight"], inputs["bias"])
    res = run_bass_kernel_spmd(nc, in_maps, core_ids=list(range(N_CORES)), **kw)
    return gather_output(res.results), res


def kernel(x, weight, bias):
    out, _ = run({"x": x, "weight": weight, "bias": bias})
    return out


# revision 22
# speedup vs baseline: 1.0706x; 1.0352x over previous
"""LocallyConnected2d Trainium2 kernel (bf16 pipeline).

Problem: out[b,o,oh,ow] = sum_{c,ki,kj} x[b,c,oh+ki,ow+kj] * W[o,oh,ow,c,ki,kj] + bias[o,oh,ow]
Shapes: x[32,32,64,64], W[64,62,62,32,3,3], bias[64,62,62] -> out[32,64,62,62], fp32 I/O.

The untied weight tensor (283 MB fp32) is read exactly once -> the kernel is
HBM-bandwidth bound. All operands ship as bf16 (accuracy gate 2e-2 vs ~2.7e-3
measured bf16 error), halving the dominant weight stream; PSUM accumulates in
fp32; the output returns as bf16 and is upcast on host.

Strategy (8 NeuronCores, sharded over output rows, 8 rows/core padded to 64):
- Per output location: 3 accumulating PE matmuls, K=97 each (chunk q = kernel
  row ki; features j=(kj,c) plus a ones-row at j=96 that carries bias on q=2).
- lhsT (stationary) = x patch columns [97,32b]: x ships unshifted [c,h,w64,b]
  (1.3 MB) and is replicated on-chip into 3 column-shifted replicas on
  partitions kj*32+c via SBUF->SBUF DMA, so every lhsT is a direct AP slice.
  Partition 96 is memset to 1.0 (carries the bias row).
- rhs (moving) = per-location weights [97,64o], streamed from HBM in
  half-row strips (11.9KB padded lines). DMA shape lessons (measured):
  SWDGE chops lines 16-way into ~744B packets (~97 GB/s); HWDGE fans a DMA
  across the 16 SDMA engines only when the line count is divisible by 16,
  else the whole transfer lands on ONE engine (~27 GB/s). So: 96-line
  feature DMAs + separate bias line, alternating between the two HWDGE
  rings (sync/scalar), all weight DMAs emitted before any out-store.
- One PSUM bank [128,512] per strip accumulates 8 location-groups (4
  locations x 32b on partitions, 64o per group in free); one DVE copy casts
  the bank into a persistent bf16 out tile; out-stores go in multi-strip
  chunks with fat per-partition lines.
"""

import numpy as np
import ml_dtypes

import concourse.bass as bass  # noqa: F401
import concourse.mybir as mybir
import concourse.tile as tile
from concourse import bacc
from concourse.bass_utils import run_bass_kernel_spmd

B, C_IN, H, W = 32, 32, 64, 64
C_OUT, OH, OW, KK = 64, 62, 62, 3
N_CORES = 8
ROWS = 8          # padded output rows per core (8*8=64 >= 62)
HALF = 31         # locations per strip (half an output row)
XH = ROWS + 2     # input rows needed per core
KP = 97           # contraction per chunk: 96 features + ones/bias row
NG = 8            # ceil(31/4) location groups per strip
SLINE = 3 * HALF * C_OUT  # 5952 weight elems per strip per j-line
WLINE = SLINE + 32        # padded line (non-contiguous source)
F32 = mybir.dt.float32
BF16 = mybir.dt.bfloat16
NP_BF16 = ml_dtypes.bfloat16

_NC_CACHE = {}


def _build_nc():
    nc = bacc.Bacc(
        "TRN2",
        target_bir_lowering=False,
        debug=False,
        enable_asserts=False,
        num_devices=N_CORES,
    )
    x_d = nc.dram_tensor("x", [C_IN, XH, W, B], BF16, kind="ExternalInput").ap()
    w_d = nc.dram_tensor(
        "w", [ROWS, 2, KP, WLINE], BF16, kind="ExternalInput"
    ).ap()
    # out layout: [p=(l4,b), strip, grp, o] - partition-major; host
    # unscrambles + upcasts
    o_d = nc.dram_tensor(
        "out", [128, ROWS * 2 * NG * C_OUT], BF16, kind="ExternalOutput"
    ).ap()

    with tile.TileContext(nc) as tc:
        with (
            tc.tile_pool(name="xpool", bufs=1) as xpool,
            tc.tile_pool(name="wpool", bufs=6) as wpool,
            tc.tile_pool(name="opool", bufs=1) as opool,
            tc.tile_pool(name="pspool", bufs=3, space="PSUM") as pspool,
        ):
            HZ = OW * B    # 1984 elems per h-row in the shifted replicas
            HZ64 = W * B   # 2048 elems per h-row unshifted
            # x ships once, unshifted; replicas built on-chip via HWDGE
            # SBUF->SBUF (saves 2.5 MB of HBM; SWDGE sb2sb measured slow —
            # Q7 descriptor emission delayed the stream by ~20us). Two
            # tiles so the first replicas depend only on the first load.
            x1a = xpool.tile([C_IN, 3 * HZ64], BF16)
            x1b = xpool.tile([C_IN, (XH - 3) * HZ64], BF16)
            x3 = xpool.tile([KP, XH * HZ], BF16)
            nc.vector.memset(x3[96:97, :], 1.0)
            xsrc = x_d.rearrange("c h w b -> c (h w b)")
            nc.sync.dma_start(out=x1a, in_=xsrc[:, 0 : 3 * HZ64])
            nc.scalar.dma_start(out=x1b, in_=xsrc[:, 3 * HZ64 :])
            x1av = x1a.rearrange("c (h z) -> c h z", z=HZ64)
            x1bv = x1b.rearrange("c (h z) -> c h z", z=HZ64)
            x3v = x3.rearrange("p (h z) -> p h z", z=HZ)

            def replicate(src, r0, r1, eng):
                # partition kj*32+c <- x[c, h, w+kj, b] for rows [r0, r1)
                for kj in range(KK):
                    eng.dma_start(
                        out=x3v[kj * 32 : (kj + 1) * 32, r0:r1, :],
                        in_=src[:, r0 - (0 if r0 < 3 else 3) : r1 - (0 if r0 < 3 else 3), kj * B : kj * B + HZ],
                    )

            replicate(x1av, 0, 3, nc.sync)  # before w0 on the sync ring

            QZ = HALF * C_OUT  # 1984, one chunk per kernel row q
            # all weight DMAs emitted up-front (wpool bufs provide the
            # streaming backpressure) so out-stores never head-of-line
            # block the weight stream on either ring
            wts = []
            for s in range(2 * ROWS):
                weng = nc.sync if s % 2 == 0 else nc.scalar
                wt_full = wpool.tile([KP, WLINE], BF16, tag="wt")
                wsrc = w_d[s // 2, s % 2]
                weng.dma_start(
                    out=wt_full[96:97, 0:SLINE], in_=wsrc[96:97, 0:SLINE]
                )
                if s < 2:
                    # split the first strip on each ring by q-chunk so its
                    # first matmuls unblock after 1/3 of the strip
                    for f0, f1 in ((0, QZ), (QZ, 2 * QZ), (2 * QZ, 3 * QZ)):
                        weng.dma_start(
                            out=wt_full[0:96, f0:f1], in_=wsrc[0:96, f0:f1]
                        )
                else:
                    weng.dma_start(
                        out=wt_full[0:96, 0:SLINE], in_=wsrc[0:96, 0:SLINE]
                    )
                wts.append(wt_full)
                if s == 1:
                    # replicas for rows 3-9 ride the scalar ring right
                    # after strip 1's weights (needed from strip 2 on)
                    replicate(x1bv, 3, XH, nc.scalar)

            SZ = NG * C_OUT  # 512 out elems per strip per partition
            ot = opool.tile([128, 2 * ROWS * SZ], BF16)  # all strips
            OUT_CHUNKS = {3: (0, 4), 7: (4, 8), 11: (8, 12), 14: (12, 15), 15: (15, 16)}
            for s in range(2 * ROWS):
                row, half = s // 2, s % 2
                wt = wts[s]
                # one PSUM bank per strip: partitions (l4,b), free (grp, o)
                ps = pspool.tile([128, SZ], F32, tag="ps")
                for g in range(NG):
                    gn = min(4, HALF - g * 4)  # 4,4,...,3
                    for li in range(4):
                        # pad slot in the last group duplicates the prior
                        # location (keeps PSUM fully written; host drops it)
                        eff = min(li, gn - 1)
                        ow = half * HALF + g * 4 + eff
                        loff = (g * 4 + eff) * C_OUT
                        for q in range(3):
                            nc.tensor.matmul(
                                ps[32 * li : 32 * li + 32, g * C_OUT : (g + 1) * C_OUT],
                                x3[
                                    :,
                                    (row + q) * HZ
                                    + ow * B : (row + q) * HZ
                                    + ow * B
                                    + B,
                                ],  # [97, 32] lhsT
                                wt[:, q * QZ + loff : q * QZ + loff + C_OUT],
                                start=(q == 0),
                                stop=(q == 2),
                                tile_position=(0, 32 * li),
                            )
                nc.vector.tensor_copy(out=ot[:, s * SZ : (s + 1) * SZ], in_=ps)
                if s in OUT_CHUNKS:
                    c0, c1 = OUT_CHUNKS[s]
                    nc.scalar.dma_start(
                        out=o_d[:, c0 * SZ : c1 * SZ], in_=ot[:, c0 * SZ : c1 * SZ]
                    )

    nc.compile()
    return nc


def get_nc():
    if "nc" not in _NC_CACHE:
        _NC_CACHE["nc"] = _build_nc()
    return _NC_CACHE["nc"]


def prep_inputs(x, weight, bias):
    """Host-side shard + layout prep. Returns per-core in_maps."""
    x = np.asarray(x, dtype=np.float32)
    weight = np.asarray(weight, dtype=np.float32)
    bias = np.asarray(bias, dtype=np.float32)

    # w_prep[oh, j=kj*32+c, q=ki, ow, o]; j=96 row: 0 for q<2, bias for q=2
    wp = np.zeros((N_CORES * ROWS, KP, 3, OW, C_OUT), NP_BF16)
    wp[:OH, :96] = (
        weight.transpose(1, 5, 3, 4, 2, 0).reshape(OH, 96, 3, OW, C_OUT)
    ).astype(NP_BF16)
    wp[:OH, 96, 2] = bias.transpose(1, 2, 0).astype(NP_BF16)
    # half-row strips with padded lines: [row, half, j, (q l o)+32]
    wp = wp.reshape(N_CORES * ROWS, KP, 3, 2, HALF, C_OUT).transpose(0, 3, 1, 2, 4, 5)
    wpad = np.zeros((N_CORES * ROWS, 2, KP, WLINE), NP_BF16)
    wpad[:, :, :, :SLINE] = wp.reshape(N_CORES * ROWS, 2, KP, SLINE)
    wp = wpad

    # x unshifted [c, h, w64, b] per core (replication happens on-chip)
    xp = np.zeros((C_IN, N_CORES * ROWS + 2, W, B), NP_BF16)
    xp[:, :H] = x.transpose(1, 2, 3, 0).astype(NP_BF16)

    in_maps = []
    for c in range(N_CORES):
        r0 = c * ROWS
        in_maps.append(
            {
                "x": np.ascontiguousarray(xp[:, r0 : r0 + XH]),
                "w": np.ascontiguousarray(wp[r0 : r0 + ROWS]),
            }
        )
    return in_maps


def gather_output(results):
    """results: list of per-core out dicts -> full [B, C_OUT, OH, OW] fp32."""
    out = np.empty((B, C_OUT, OH, OW), np.float32)
    for c in range(N_CORES):
        # out[p=(l4,b), (strip, grp, o)]
        oc = np.asarray(results[c]["out"]).astype(np.float32)
        v = oc.reshape(4, B, ROWS, 2, NG, C_OUT)
        # ow = half*31 + grp*4 + l  (grp*4+l < 31)
        arr = v.transpose(1, 5, 2, 3, 4, 0).reshape(B, C_OUT, ROWS, 2, 32)
        arr = arr[:, :, :, :, :HALF].reshape(B, C_OUT, ROWS, OW)
        r0 = c * ROWS
        rows = min(ROWS, OH - r0)
        out[:, :, r0 : r0 + rows, :] = arr[:, :, :rows, :]
    return out


def run(inputs, **kw):
    nc = get_nc()
    in_maps = prep_inputs(inputs["x"], inputs["weight"], inputs["bias"])
    res = run_bass_kernel_spmd(nc, in_maps, core_ids=list(range(N_CORES)), **kw)
    return gather_output(res.results), res


def kernel(x, weight, bias):
    out, _ = run({"x": x, "weight": weight, "bias": bias})
    return out


# revision 25
# speedup vs baseline: 1.0954x; 1.0232x over previous
"""LocallyConnected2d Trainium2 kernel (bf16 pipeline).

Problem: out[b,o,oh,ow] = sum_{c,ki,kj} x[b,c,oh+ki,ow+kj] * W[o,oh,ow,c,ki,kj] + bias[o,oh,ow]
Shapes: x[32,32,64,64], W[64,62,62,32,3,3], bias[64,62,62] -> out[32,64,62,62], fp32 I/O.

The untied weight tensor (283 MB fp32) is read exactly once -> the kernel is
HBM-bandwidth bound. All operands ship as bf16 (accuracy gate 2e-2 vs ~2.7e-3
measured bf16 error), halving the dominant weight stream; PSUM accumulates in
fp32; the output returns as bf16 and is upcast on host.

Strategy (8 NeuronCores, sharded over output rows, 8 rows/core padded to 64):
- Per output location: 3 accumulating PE matmuls, K=97 each (chunk q = kernel
  row ki; features j=(kj,c) plus a ones-row at j=96 that carries bias on q=2).
- lhsT (stationary) = x patch columns [97,32b]: x ships unshifted [c,h,w64,b]
  (1.3 MB) and is replicated on-chip into 3 column-shifted replicas on
  partitions kj*32+c via SBUF->SBUF DMA, so every lhsT is a direct AP slice.
  Partition 96 is memset to 1.0 (carries the bias row).
- rhs (moving) = per-location weights [97,64o], streamed from HBM in
  half-row strips (11.9KB padded lines). DMA shape lessons (measured):
  SWDGE chops lines 16-way into ~744B packets (~97 GB/s); HWDGE fans a DMA
  across the 16 SDMA engines only when the line count is divisible by 16,
  else the whole transfer lands on ONE engine (~27 GB/s). So: 96-line
  feature DMAs + separate bias line, alternating between the two HWDGE
  rings (sync/scalar), all weight DMAs emitted before any out-store.
- One PSUM bank [128,512] per strip accumulates 8 location-groups (4
  locations x 32b on partitions, 64o per group in free); one DVE copy casts
  the bank into a persistent bf16 out tile; out-stores go in multi-strip
  chunks with fat per-partition lines.
"""

import numpy as np
import ml_dtypes

import concourse.bass as bass  # noqa: F401
import concourse.mybir as mybir
import concourse.tile as tile
from concourse import bacc
from concourse.bass_utils import run_bass_kernel_spmd

B, C_IN, H, W = 32, 32, 64, 64
C_OUT, OH, OW, KK = 64, 62, 62, 3
N_CORES = 8
ROWS = 8          # padded output rows per core (8*8=64 >= 62)
HALF = 31         # locations per strip (half an output row)
XH = ROWS + 2     # input rows needed per core
KP = 97           # contraction per chunk: 96 features + ones/bias row
NG = 8            # ceil(31/4) location groups per strip
SLINE = 3 * HALF * C_OUT  # 5952 weight elems per strip per j-line
WLINE = SLINE + 32        # padded line (non-contiguous source)
F32 = mybir.dt.float32
BF16 = mybir.dt.bfloat16
NP_BF16 = ml_dtypes.bfloat16

_NC_CACHE = {}


def _build_nc():
    nc = bacc.Bacc(
        "TRN2",
        target_bir_lowering=False,
        debug=False,
        enable_asserts=False,
        num_devices=N_CORES,
    )
    # x ships host-transposed AND pre-shifted into 3 kj-replicas
    # [kj, c, h, w(62), b]: on-chip replication was tried (SWDGE and HWDGE
    # sb2sb) and lost — sb2sb consumes the same per-engine descriptor
    # cadence that bounds the HBM stream, and delays the first strips.
    x_d = nc.dram_tensor("x", [KK, C_IN, XH, OW, B], BF16, kind="ExternalInput").ap()
    w_d = nc.dram_tensor(
        "w", [ROWS, 2, KP, WLINE], BF16, kind="ExternalInput"
    ).ap()
    # out layout: [p=(l4,b), strip, grp, o] - partition-major; host
    # unscrambles + upcasts
    o_d = nc.dram_tensor(
        "out", [128, ROWS * 2 * NG * C_OUT], BF16, kind="ExternalOutput"
    ).ap()

    with tile.TileContext(nc) as tc:
        with (
            tc.tile_pool(name="xpool", bufs=1) as xpool,
            tc.tile_pool(name="wpool", bufs=6) as wpool,
            tc.tile_pool(name="opool", bufs=1) as opool,
            tc.tile_pool(name="pspool", bufs=3, space="PSUM") as pspool,
        ):
            HZ = OW * B  # 1984 elems per h-row
            x3 = xpool.tile([KP, XH * HZ], BF16)
            # partition 96 = 1.0 (carries the bias row); memset instead of a
            # DRAM ones-load — the ones DMA previously gated the first
            # matmuls behind megabytes of x on the same ring
            nc.vector.memset(x3[96:97, :], 1.0)
            xsrc = x_d.rearrange("k c h w b -> (k c) (h w b)")

            def load_x_rows(r0, r1, eng):
                eng.dma_start(
                    out=x3[0:96, r0 * HZ : r1 * HZ],
                    in_=xsrc[0:96, r0 * HZ : r1 * HZ],
                )

            # all x on the sync ring (scalar carries the out-stores): rows
            # 0-2 ahead of w0, the rest interleaved behind the first strips
            load_x_rows(0, 3, nc.sync)

            QZ = HALF * C_OUT  # 1984, one chunk per kernel row q
            # all weight DMAs emitted up-front (wpool bufs provide the
            # streaming backpressure) so out-stores never head-of-line
            # block the weight stream on either ring
            wts = []
            for s in range(2 * ROWS):
                weng = nc.sync if s % 2 == 0 else nc.scalar
                wt_full = wpool.tile([KP, WLINE], BF16, tag="wt")
                wsrc = w_d[s // 2, s % 2]
                weng.dma_start(
                    out=wt_full[96:97, 0:SLINE], in_=wsrc[96:97, 0:SLINE]
                )
                if s < 2:
                    # split the first strip on each ring by q-chunk so its
                    # first matmuls unblock after 1/3 of the strip
                    for f0, f1 in ((0, QZ), (QZ, 2 * QZ), (2 * QZ, 3 * QZ)):
                        weng.dma_start(
                            out=wt_full[0:96, f0:f1], in_=wsrc[0:96, f0:f1]
                        )
                else:
                    weng.dma_start(
                        out=wt_full[0:96, 0:SLINE], in_=wsrc[0:96, 0:SLINE]
                    )
                wts.append(wt_full)
                if s == 0:
                    load_x_rows(3, 6, nc.sync)
                elif s == 2:
                    load_x_rows(6, 10, nc.sync)

            SZ = NG * C_OUT  # 512 out elems per strip per partition
            ot = opool.tile([128, 2 * ROWS * SZ], BF16)  # all strips
            OUT_CHUNKS = {3: (0, 4), 7: (4, 8), 11: (8, 12), 14: (12, 15), 15: (15, 16)}
            for s in range(2 * ROWS):
                row, half = s // 2, s % 2
                wt = wts[s]
                # one PSUM bank per strip: partitions (l4,b), free (grp, o)
                ps = pspool.tile([128, SZ], F32, tag="ps")
                for g in range(NG):
                    gn = min(4, HALF - g * 4)  # 4,4,...,3
                    for li in range(4):
                        # pad slot in the last group duplicates the prior
                        # location (keeps PSUM fully written; host drops it)
                        eff = min(li, gn - 1)
                        ow = half * HALF + g * 4 + eff
                        loff = (g * 4 + eff) * C_OUT
                        for q in range(3):
                            nc.tensor.matmul(
                                ps[32 * li : 32 * li + 32, g * C_OUT : (g + 1) * C_OUT],
                                x3[
                                    :,
                                    (row + q) * HZ
                                    + ow * B : (row + q) * HZ
                                    + ow * B
                                    + B,
                                ],  # [97, 32] lhsT
                                wt[:, q * QZ + loff : q * QZ + loff + C_OUT],
                                start=(q == 0),
                                stop=(q == 2),
                                tile_position=(0, 32 * li),
                            )
                nc.vector.tensor_copy(out=ot[:, s * SZ : (s + 1) * SZ], in_=ps)
                if s in OUT_CHUNKS:
                    c0, c1 = OUT_CHUNKS[s]
                    nc.scalar.dma_start(
                        out=o_d[:, c0 * SZ : c1 * SZ], in_=ot[:, c0 * SZ : c1 * SZ]
                    )

    nc.compile()
    return nc


def get_nc():
    if "nc" not in _NC_CACHE:
        _NC_CACHE["nc"] = _build_nc()
    return _NC_CACHE["nc"]


def prep_inputs(x, weight, bias):
    """Host-side shard + layout prep. Returns per-core in_maps."""
    x = np.asarray(x, dtype=np.float32)
    weight = np.asarray(weight, dtype=np.float32)
    bias = np.asarray(bias, dtype=np.float32)

    # w_prep[oh, j=kj*32+c, q=ki, ow, o]; j=96 row: 0 for q<2, bias for q=2
    wp = np.zeros((N_CORES * ROWS, KP, 3, OW, C_OUT), NP_BF16)
    wp[:OH, :96] = (
        weight.transpose(1, 5, 3, 4, 2, 0).reshape(OH, 96, 3, OW, C_OUT)
    ).astype(NP_BF16)
    wp[:OH, 96, 2] = bias.transpose(1, 2, 0).astype(NP_BF16)
    # half-row strips with padded lines: [row, half, j, (q l o)+32]
    wp = wp.reshape(N_CORES * ROWS, KP, 3, 2, HALF, C_OUT).transpose(0, 3, 1, 2, 4, 5)
    wpad = np.zeros((N_CORES * ROWS, 2, KP, WLINE), NP_BF16)
    wpad[:, :, :, :SLINE] = wp.reshape(N_CORES * ROWS, 2, KP, SLINE)
    wp = wpad

    # x pre-shifted into 3 kj-replicas [kj, c, h, w(62), b]
    xp = np.zeros((B, C_IN, N_CORES * ROWS + 2, W), NP_BF16)
    xp[:, :, :H] = x.astype(NP_BF16)
    xt = xp.transpose(1, 2, 3, 0)  # [c, h, w, b]

    in_maps = []
    for c in range(N_CORES):
        r0 = c * ROWS
        xc = xt[:, r0 : r0 + XH]  # [c, 10, 64, b]
        xsh = np.stack([xc[:, :, kj : kj + OW, :] for kj in range(KK)])
        in_maps.append(
            {
                "x": np.ascontiguousarray(xsh),
                "w": np.ascontiguousarray(wp[r0 : r0 + ROWS]),
            }
        )
    return in_maps


def gather_output(results):
    """results: list of per-core out dicts -> full [B, C_OUT, OH, OW] fp32."""
    out = np.empty((B, C_OUT, OH, OW), np.float32)
    for c in range(N_CORES):
        # out[p=(l4,b), (strip, grp, o)]
        oc = np.asarray(results[c]["out"]).astype(np.float32)
        v = oc.reshape(4, B, ROWS, 2, NG, C_OUT)
        # ow = half*31 + grp*4 + l  (grp*4+l < 31)
        arr = v.transpose(1, 5, 2, 3, 4, 0).reshape(B, C_OUT, ROWS, 2, 32)
        arr = arr[:, :, :, :, :HALF].reshape(B, C_OUT, ROWS, OW)
        r0 = c * ROWS
        rows = min(ROWS, OH - r0)
        out[:, :, r0 : r0 + rows, :] = arr[:, :, :rows, :]
    return out


def run(inputs, **kw):
    nc = get_nc()
    in_maps = prep_inputs(inputs["x"], inputs["weight"], inputs["bias"])
    res = run_bass_kernel_spmd(nc, in_maps, core_ids=list(range(N_CORES)), **kw)
    return gather_output(res.results), res


def kernel(x, weight, bias):
    out, _ = run({"x": x, "weight": weight, "bias": bias})
    return out


# revision 28
# speedup vs baseline: 1.1165x; 1.0192x over previous
"""LocallyConnected2d Trainium2 kernel (bf16 pipeline).

Problem: out[b,o,oh,ow] = sum_{c,ki,kj} x[b,c,oh+ki,ow+kj] * W[o,oh,ow,c,ki,kj] + bias[o,oh,ow]
Shapes: x[32,32,64,64], W[64,62,62,32,3,3], bias[64,62,62] -> out[32,64,62,62], fp32 I/O.

The untied weight tensor (283 MB fp32) is read exactly once -> the kernel is
HBM-bandwidth bound. All operands ship as bf16 (accuracy gate 2e-2 vs ~2.7e-3
measured bf16 error), halving the dominant weight stream; PSUM accumulates in
fp32; the output returns as bf16 and is upcast on host.

Strategy (8 NeuronCores, sharded over output rows, 8 rows/core padded to 64):
- Per output location: 3 accumulating PE matmuls, K=97 each (chunk q = kernel
  row ki; features j=(kj,c) plus a ones-row at j=96 that carries bias on q=2).
- lhsT (stationary) = x patch columns [97,32b]: x ships unshifted [c,h,w64,b]
  (1.3 MB) and is replicated on-chip into 3 column-shifted replicas on
  partitions kj*32+c via SBUF->SBUF DMA, so every lhsT is a direct AP slice.
  Partition 96 is memset to 1.0 (carries the bias row).
- rhs (moving) = per-location weights [97,64o], streamed from HBM in
  half-row strips (11.9KB padded lines). DMA shape lessons (measured):
  SWDGE chops lines 16-way into ~744B packets (~97 GB/s); HWDGE fans a DMA
  across the 16 SDMA engines only when the line count is divisible by 16,
  else the whole transfer lands on ONE engine (~27 GB/s). So: 96-line
  feature DMAs + separate bias line, alternating between the two HWDGE
  rings (sync/scalar), all weight DMAs emitted before any out-store.
- One PSUM bank [128,512] per strip accumulates 8 location-groups (4
  locations x 32b on partitions, 64o per group in free); one DVE copy casts
  the bank into a persistent bf16 out tile; out-stores go in multi-strip
  chunks with fat per-partition lines.
"""

import numpy as np
import ml_dtypes

import concourse.bass as bass  # noqa: F401
import concourse.mybir as mybir
import concourse.tile as tile
from concourse import bacc
from concourse.bass_utils import run_bass_kernel_spmd

B, C_IN, H, W = 32, 32, 64, 64
C_OUT, OH, OW, KK = 64, 62, 62, 3
N_CORES = 8
ROWS = 8          # padded output rows per core (8*8=64 >= 62)
HALF = 31         # locations per strip (half an output row)
XH = ROWS + 2     # input rows needed per core
KP = 97           # contraction per chunk: 96 features + ones/bias row
NG = 8            # ceil(31/4) location groups per strip
SLINE = 3 * HALF * C_OUT  # 5952 weight elems per strip per j-line
WLINE = SLINE + 32        # padded line (non-contiguous source)
F32 = mybir.dt.float32
BF16 = mybir.dt.bfloat16
NP_BF16 = ml_dtypes.bfloat16

_NC_CACHE = {}


def _build_nc():
    nc = bacc.Bacc(
        "TRN2",
        target_bir_lowering=False,
        debug=False,
        enable_asserts=False,
        num_devices=N_CORES,
    )
    # x ships host-transposed AND pre-shifted into 3 kj-replicas
    # [kj, c, h, w(62), b]: on-chip replication was tried (SWDGE and HWDGE
    # sb2sb) and lost — sb2sb consumes the same per-engine descriptor
    # cadence that bounds the HBM stream, and delays the first strips.
    x_d = nc.dram_tensor("x", [KK, C_IN, XH, OW, B], BF16, kind="ExternalInput").ap()
    w_d = nc.dram_tensor(
        "w", [ROWS, 2, KP, WLINE], BF16, kind="ExternalInput"
    ).ap()
    # out layout: [p=(l4,b), strip, grp, o] - partition-major; host
    # unscrambles + upcasts
    o_d = nc.dram_tensor(
        "out", [128, ROWS * 2 * NG * C_OUT], BF16, kind="ExternalOutput"
    ).ap()

    with tile.TileContext(nc) as tc:
        with (
            tc.tile_pool(name="xpool", bufs=1) as xpool,
            tc.tile_pool(name="wpool", bufs=6) as wpool,
            tc.tile_pool(name="opool", bufs=1) as opool,
            tc.tile_pool(name="pspool", bufs=3, space="PSUM") as pspool,
        ):
            HZ = OW * B  # 1984 elems per h-row
            # x in THREE row-range tiles: matmul waits proved to be
            # tile-granular, so with one x3 tile the first strips waited on
            # x rows they never read. Each matmul touches exactly one h-row
            # (row+q), so row-range tiles split cleanly.
            XROWS = ((0, 3), (3, 6), (6, XH))
            x3s = [
                xpool.tile([KP, (r1 - r0) * HZ], BF16, name=f"x3_{r0}")
                for r0, r1 in XROWS
            ]
            for t in x3s:
                # partition 96 = 1.0 (carries the bias row); memset instead
                # of a DRAM ones-load
                nc.vector.memset(t[96:97, :], 1.0)
            xsrc = x_d.rearrange("k c h w b -> (k c) (h w b)")

            def xrow(r):
                # (tile, base) for input row r
                for ti, (r0, r1) in enumerate(XROWS):
                    if r < r1:
                        return x3s[ti], (r - r0) * HZ
                raise AssertionError

            def load_x_rows(ti, eng):
                r0, r1 = XROWS[ti]
                eng.dma_start(
                    out=x3s[ti][0:96, :],
                    in_=xsrc[0:96, r0 * HZ : r1 * HZ],
                )

            # x rides the scalar ring (ahead of the odd strips' weights) so
            # w0 starts streaming on sync immediately
            load_x_rows(0, nc.scalar)

            QZ = HALF * C_OUT  # 1984, one chunk per kernel row q
            # all weight DMAs emitted up-front (wpool bufs provide the
            # streaming backpressure) so out-stores never head-of-line
            # block the weight stream on either ring. Strips 13/15 join the
            # sync ring to offset scalar's x + out-store load.
            wts = []
            for s in range(2 * ROWS):
                weng = nc.sync if (s % 2 == 0 or s >= 13) else nc.scalar
                wt_full = wpool.tile([KP, WLINE], BF16, tag="wt")
                wsrc = w_d[s // 2, s % 2]
                weng.dma_start(
                    out=wt_full[96:97, 0:SLINE], in_=wsrc[96:97, 0:SLINE]
                )
                if s < 2:
                    # split the first strip on each ring by q-chunk so its
                    # first matmuls unblock after 1/3 of the strip
                    for f0, f1 in ((0, QZ), (QZ, 2 * QZ), (2 * QZ, 3 * QZ)):
                        weng.dma_start(
                            out=wt_full[0:96, f0:f1], in_=wsrc[0:96, f0:f1]
                        )
                else:
                    weng.dma_start(
                        out=wt_full[0:96, 0:SLINE], in_=wsrc[0:96, 0:SLINE]
                    )
                wts.append(wt_full)
                if s == 1:
                    load_x_rows(1, nc.scalar)
                elif s == 3:
                    load_x_rows(2, nc.scalar)

            SZ = NG * C_OUT  # 512 out elems per strip per partition
            ot = opool.tile([128, 2 * ROWS * SZ], BF16)  # all strips
            OUT_CHUNKS = {3: (0, 4), 7: (4, 8), 11: (8, 12), 14: (12, 15), 15: (15, 16)}
            for s in range(2 * ROWS):
                row, half = s // 2, s % 2
                wt = wts[s]
                # one PSUM bank per strip: partitions (l4,b), free (grp, o)
                ps = pspool.tile([128, SZ], F32, tag="ps")
                for g in range(NG):
                    gn = min(4, HALF - g * 4)  # 4,4,...,3
                    for li in range(4):
                        # pad slot in the last group duplicates the prior
                        # location (keeps PSUM fully written; host drops it)
                        eff = min(li, gn - 1)
                        ow = half * HALF + g * 4 + eff
                        loff = (g * 4 + eff) * C_OUT
                        for q in range(3):
                            xt, xbase = xrow(row + q)
                            nc.tensor.matmul(
                                ps[32 * li : 32 * li + 32, g * C_OUT : (g + 1) * C_OUT],
                                xt[
                                    :, xbase + ow * B : xbase + ow * B + B
                                ],  # [97, 32] lhsT
                                wt[:, q * QZ + loff : q * QZ + loff + C_OUT],
                                start=(q == 0),
                                stop=(q == 2),
                                tile_position=(0, 32 * li),
                            )
                nc.vector.tensor_copy(out=ot[:, s * SZ : (s + 1) * SZ], in_=ps)
                if s in OUT_CHUNKS:
                    c0, c1 = OUT_CHUNKS[s]
                    nc.scalar.dma_start(
                        out=o_d[:, c0 * SZ : c1 * SZ], in_=ot[:, c0 * SZ : c1 * SZ]
                    )

    nc.compile()
    return nc


def get_nc():
    if "nc" not in _NC_CACHE:
        _NC_CACHE["nc"] = _build_nc()
    return _NC_CACHE["nc"]


def prep_inputs(x, weight, bias):
    """Host-side shard + layout prep. Returns per-core in_maps."""
    x = np.asarray(x, dtype=np.float32)
    weight = np.asarray(weight, dtype=np.float32)
    bias = np.asarray(bias, dtype=np.float32)

    # w_prep[oh, j=kj*32+c, q=ki, ow, o]; j=96 row: 0 for q<2, bias for q=2
    wp = np.zeros((N_CORES * ROWS, KP, 3, OW, C_OUT), NP_BF16)
    wp[:OH, :96] = (
        weight.transpose(1, 5, 3, 4, 2, 0).reshape(OH, 96, 3, OW, C_OUT)
    ).astype(NP_BF16)
    wp[:OH, 96, 2] = bias.transpose(1, 2, 0).astype(NP_BF16)
    # half-row strips with padded lines: [row, half, j, (q l o)+32]
    wp = wp.reshape(N_CORES * ROWS, KP, 3, 2, HALF, C_OUT).transpose(0, 3, 1, 2, 4, 5)
    wpad = np.zeros((N_CORES * ROWS, 2, KP, WLINE), NP_BF16)
    wpad[:, :, :, :SLINE] = wp.reshape(N_CORES * ROWS, 2, KP, SLINE)
    wp = wpad

    # x pre-shifted into 3 kj-replicas [kj, c, h, w(62), b]
    xp = np.zeros((B, C_IN, N_CORES * ROWS + 2, W), NP_BF16)
    xp[:, :, :H] = x.astype(NP_BF16)
    xt = xp.transpose(1, 2, 3, 0)  # [c, h, w, b]

    in_maps = []
    for c in range(N_CORES):
        r0 = c * ROWS
        xc = xt[:, r0 : r0 + XH]  # [c, 10, 64, b]
        xsh = np.stack([xc[:, :, kj : kj + OW, :] for kj in range(KK)])
        in_maps.append(
            {
                "x": np.ascontiguousarray(xsh),
                "w": np.ascontiguousarray(wp[r0 : r0 + ROWS]),
            }
        )
    return in_maps


def gather_output(results):
    """results: list of per-core out dicts -> full [B, C_OUT, OH, OW] fp32."""
    out = np.empty((B, C_OUT, OH, OW), np.float32)
    for c in range(N_CORES):
        # out[p=(l4,b), (strip, grp, o)]
        oc = np.asarray(results[c]["out"]).astype(np.float32)
        v = oc.reshape(4, B, ROWS, 2, NG, C_OUT)
        # ow = half*31 + grp*4 + l  (grp*4+l < 31)
        arr = v.transpose(1, 5, 2, 3, 4, 0).reshape(B, C_OUT, ROWS, 2, 32)
        arr = arr[:, :, :, :, :HALF].reshape(B, C_OUT, ROWS, OW)
        r0 = c * ROWS
        rows = min(ROWS, OH - r0)
        out[:, :, r0 : r0 + rows, :] = arr[:, :, :rows, :]
    return out


def run(inputs, **kw):
    nc = get_nc()
    in_maps = prep_inputs(inputs["x"], inputs["weight"], inputs["bias"])
    res = run_bass_kernel_spmd(nc, in_maps, core_ids=list(range(N_CORES)), **kw)
    return gather_output(res.results), res


def kernel(x, weight, bias):
    out, _ = run({"x": x, "weight": weight, "bias": bias})
    return out
